# revision 21
# baseline (speedup 1.0000x reference)
"""Trainium2 Bass kernel for nn_Attention (sparse_attention, T=3) — v6.

Math (per batch row b, derived from the reference):
    zq = z[:, :3*2048].reshape(B, 3, D)   (q and v source)
    zk = z[:, 3*2048:].reshape(B, 3, D)
    scores[t,s] = (zq[t] @ M~ @ zk[s] + zq[t].a + r.zk[s] + kap)
      with M~ = SQD wq.T wk, a = SQD wq.T bk, r = SQD bq wk, kap = SQD bq.bk
    strictly-lower entries of scores are replaced by 0 before softmax
    p = softmax(scores); w[s] = sum_t p[t,s]
    y = (sum_s w[s]*zq[s]) @ Wz + c0,  Wz = wv.T wo.T, c0 = 3*bv@wo.T + 3*bo

v6 design vs v5:
  - wq/wk/wkr live in their own leftmost pool released right after the M/r
    matmuls, so the phase-2 z loads (placed in a manually-managed pool) can
    start ~50us earlier instead of waiting for all of prep to free SBUF.
  - a/kap partition reductions moved from gpsimd.partition_all_reduce to PE
    ones-matmuls: the gpsimd queue must stay free so AG1b is issued
    immediately after AG1a (collectives serialize with ~40us latency each).
  - a/r broadcasts reordered after the AG2 issue.
  - phase 4's first wz quarter is prefetched right after the last G matmul
    (m_pool released early) so the PE does not stall at the phase boundary.

v5 recap: host pre-transposed/fp8 zq^T (no PE transposes); r folded into the
G psum evacuation; row-0 bias dot dropped; c0 as a K=1 matmul; zv^T via
DRAM round trip through the DMA XBAR transpose.
"""

import sys

sys.path.insert(0, "/opt/trn_rl_repo")

import ml_dtypes
import numpy as np
from concourse import bacc, bass, masks, mybir, tile
from concourse.bass_utils import run_bass_kernel_spmd

F32 = mybir.dt.float32
BF16 = mybir.dt.bfloat16
F8 = mybir.dt.float8e4
ADD = mybir.AluOpType.add
MULT = mybir.AluOpType.mult
CPY = mybir.ActivationFunctionType.Copy
EXP = mybir.ActivationFunctionType.Exp

B = 8192
D = 2048
T = 3
NCORES = 8
DC = D // 128      # 16 d-chunks
EC = D // 512      # 4 e-chunks (512-wide psum banks)
SH = D // NCORES   # 256 rows of M/Wz owned per core
SQD = 1.0 / float(np.sqrt(np.float32(D)))
BF = ml_dtypes.bfloat16
F8E = ml_dtypes.float8_e4m3fn
LAM = 2048.0       # fp8 scale for M~/a/r (raw values underflow e4m3)
# m_bf stores global d-chunk 2c+h at slot h*8+c: slots 0..7 arrive with the
# first gather half (AG1a), 8..15 with the second. zqt8 uses the same slot
# order so DoubleRow pairs (2u, 2u+1) are contiguous in both operands.
D_SLOT_GLOBAL = [2 * (s % 8) + s // 8 for s in range(DC)]
DR = mybir.MatmulPerfMode.DoubleRow


def emit(tc, aps, b_loc):
    nc = tc.nc
    z, zqt8 = aps["z"], aps["zqt8"]
    wq_s, wk, wv_s, woT = aps["wq_s"], aps["wk"], aps["wv_s"], aps["woT"]
    bq, bk, bv, bo, out = aps["bq"], aps["bk"], aps["bv"], aps["bo"], aps["out"]
    NB = b_loc // 128

    const = tc.alloc_tile_pool(name="const", bufs=1)
    persist = tc.alloc_tile_pool(name="persist", bufs=1)

    # bias columns: col[p, c] = vec[c*128 + p]
    bq_col = const.tile([128, DC], F32)
    bk_col = const.tile([128, DC], F32)
    bv_col = const.tile([128, DC], F32)
    bo_row = const.tile([1, D], F32)
    nc.gpsimd.dma_start(bq_col[:], bq.rearrange("(c p) -> p c", p=128))
    nc.gpsimd.dma_start(bk_col[:], bk.rearrange("(c p) -> p c", p=128))
    nc.gpsimd.dma_start(bv_col[:], bv.rearrange("(c p) -> p c", p=128))
    nc.gpsimd.dma_start(bo_row[:], bo[None, :])

    bq_colbf = const.tile([128, DC], BF16)
    nc.vector.tensor_copy(bq_colbf[:], bq_col[:])
    ones_col = const.tile([1, 128], BF16)
    nc.vector.memset(ones_col[:], 1.0)
    ones_128 = const.tile([128, 1], BF16)
    nc.vector.memset(ones_128[:], 1.0)

    a_rep = persist.tile([128, D], BF16)    # SQD * wq.T @ bk, bcast rows
    r_rep = persist.tile([128, D], BF16)    # SQD * bq @ wk, bcast rows
    c0_row = persist.tile([1, D], BF16)     # 3*bv @ wo.T + 3*bo
    kap_col = persist.tile([128, 1], F32)   # SQD * bq.bk
    kap_row = persist.tile([1, 1], F32)
    a_row8 = persist.tile([1, D], F8)
    r_row8 = persist.tile([1, D], F8)

    # gather buffers (DRAM); M gather split into two 128-row halves
    # ag1a rows: 0 = a-slice, 1 = r-slice, 2..129 = M~ dd0 rows
    ag1a_in = nc.dram_tensor("ag1a_in", [130, D], F8).ap()
    ag1a_out = nc.dram_tensor("ag1a_out", [NCORES, 130, D], F8,
                              addr_space="Shared").ap()
    ag1b_in = nc.dram_tensor("ag1b_in", [128, D], F8).ap()
    ag1b_out = nc.dram_tensor("ag1b_out", [NCORES, 128, D], F8,
                              addr_space="Shared").ap()
    ag2_in = nc.dram_tensor("ag2_in", [SH, D], BF16).ap()
    ag2_out = nc.dram_tensor("ag2_out", [NCORES, SH, D], BF16,
                             addr_space="Shared").ap()
    zv_dram = nc.dram_tensor("zv_dram", [b_loc, D], BF16).ap()

    zvT_pool = tc.alloc_tile_pool(name="zvT_pool", bufs=1, side="right")
    zvT_all = zvT_pool.tile([128, DC, b_loc], BF16)  # zv^T[d, b]
    # weight pool on the right stack, released right after the M/r matmuls
    # so the phase-2 z pool can reuse the space early (pools are LIFO)
    p_wk = tc.alloc_tile_pool(name="p0_wk", bufs=1, side="right")

    RG = [list(range(NCORES))]
    state = [None] * NB
    p_z_box = [None]

    def sec_a(ib):
        """input loads for tile ib"""
        p_z = p_z_box[0]
        r0 = ib * 128
        st = {}
        st["zqt"] = p_z.tile([128, T, DC, 128], F8, tag="zqt", bufs=2,
                             name="zqt")
        nc.sync.dma_start(st["zqt"][:], zqt8[ib])
        st["zq"] = p_z.tile([128, T, D], BF16, tag="zq", bufs=2, name="zq")
        nc.scalar.dma_start(st["zq"][:], z[r0:r0 + 128, 0:T * D])
        st["zk"] = p_z.tile([128, T, D], BF16, tag="zk", bufs=2, name="zk")
        nc.scalar.dma_start(st["zk"][:], z[r0:r0 + 128, T * D:2 * T * D])
        return st

    # ---------------- Phase 0: M~/Wz slices + r/a/c0/kap -------------------
    with (
        tc.tile_pool(name="p0_io", bufs=2) as p_io,
        tc.tile_pool(name="p0_acc", bufs=1) as p_acc,
        tc.tile_pool(name="p0_ps", bufs=1, space="PSUM") as pp,
    ):
        # chunked contiguous weight loads, spread over queues
        wq_sb = p_wk.tile([128, DC, SH], BF16, tag="wq")
        wk_sb = p_wk.tile([128, DC, D], BF16, tag="wk")
        wkr_sb = p_wk.tile([128, DC, SH], BF16, tag="wkr")
        wv_sb = p_acc.tile([128, DC, SH], BF16, tag="wv")
        nc.sync.dma_start(wq_sb[:], wq_s.rearrange("(c p) d -> p c d", p=128))
        for i in range(DC):
            eng = nc.scalar if i % 2 == 0 else nc.sync
            eng.dma_start(wk_sb[:, i, :], wk[i * 128:(i + 1) * 128, :])
        nc.gpsimd.dma_start(wkr_sb[:],
                            aps["wk_rs"].rearrange("(c p) d -> p c d", p=128))
        nc.gpsimd.dma_start(wv_sb[:], wv_s.rearrange("(c p) d -> p c d", p=128))

        ps_m = [pp.tile([128, 512], F32, tag=f"m{k}", name=f"ps_m{k}")
                for k in range(8)]

        # a partial (own d-slice) on DVE: a[d] = sum_i wq[i, d] bk[i];
        # partition reduction via a PE ones-matmul (gpsimd queue must stay
        # free for the back-to-back collective issues)
        aacc = p_acc.tile([128, SH], BF16, tag="aacc")
        for i in range(DC):
            if i == 0:
                nc.vector.tensor_scalar(aacc[:], wq_sb[:, 0, :],
                                        bk_col[:, 0:1], None, op0=MULT)
            else:
                at = p_acc.tile([128, SH], BF16, tag="at", bufs=2)
                nc.vector.tensor_scalar(at[:], wq_sb[:, i, :],
                                        bk_col[:, i:i + 1], None, op0=MULT)
                nc.vector.tensor_tensor(aacc[:], aacc[:], at[:], op=ADD)
        # kap inputs on DVE
        kt = p_acc.tile([128, DC], F32, tag="kt")
        nc.vector.tensor_tensor(kt[:], bq_col[:], bk_col[:], op=MULT)
        k1 = p_acc.tile([128, 1], F32, tag="k1")
        nc.vector.tensor_reduce(k1[:], kt[:], axis=mybir.AxisListType.X,
                                op=ADD)
        k1b = p_acc.tile([128, 1], BF16, tag="k1b")
        nc.vector.tensor_copy(k1b[:], k1[:])

        # r e-slice on PE: r_c = SQD * bq @ wk[:, own 256 cols] (host-sliced)
        ps_r = pp.tile([1, SH], F32, tag="m0", name="ps_r")
        for i in range(DC):
            nc.tensor.matmul(ps_r[:], bq_colbf[:, i:i + 1], wkr_sb[:, i, :],
                             start=(i == 0), stop=(i == DC - 1))
        r_loc = p_acc.tile([1, SH], F8, tag="rloc")
        nc.scalar.activation(r_loc[:], ps_r[:], CPY, scale=SQD * LAM)
        nc.sync.dma_start(ag1a_in[1:2, 0:SH], r_loc[:])

        # a partition-reduce + kap on PE (banks m4/m5: free until M dd1)
        ps_a = pp.tile([1, SH], F32, tag="m4", name="ps_a")
        nc.tensor.matmul(ps_a[:], ones_128[:], aacc[:], start=True, stop=True)
        a_loc = p_acc.tile([1, SH], F8, tag="aloc")
        nc.scalar.activation(a_loc[:], ps_a[:], CPY, scale=SQD * LAM)
        nc.sync.dma_start(ag1a_in[0:1, 0:SH], a_loc[:])
        ps_k = pp.tile([1, 1], F32, tag="m5", name="ps_k")
        nc.tensor.matmul(ps_k[:], ones_128[:], k1b[:], start=True, stop=True)
        nc.scalar.activation(kap_row[:], ps_k[:], CPY, scale=SQD)

        # M~ slice in two 128-row halves, each gathered separately
        for dd in range(2):
            for i in range(DC):
                for e in range(EC):
                    nc.tensor.matmul(
                        ps_m[dd * EC + e][:],
                        wq_sb[:, i, dd * 128:(dd + 1) * 128],
                        wk_sb[:, i, e * 512:(e + 1) * 512],
                        start=(i == 0), stop=(i == DC - 1))
            m_stage = p_acc.tile([128, D], F8, tag="stage", bufs=2,
                                 name="m_stage")
            for e in range(EC):
                nc.scalar.activation(m_stage[:, e * 512:(e + 1) * 512],
                                     ps_m[dd * EC + e][:], CPY,
                                     scale=SQD * LAM)
            if dd == 0:
                nc.sync.dma_start(ag1a_in[2:130, :], m_stage[:])
                nc.gpsimd.collective_compute(
                    "AllGather", mybir.AluOpType.bypass, replica_groups=RG,
                    ins=[ag1a_in], outs=[ag1a_out])
            else:
                nc.sync.dma_start(ag1b_in[:, :], m_stage[:])
                nc.gpsimd.collective_compute(
                    "AllGather", mybir.AluOpType.bypass, replica_groups=RG,
                    ins=[ag1b_in], outs=[ag1b_out])

        # wq/wk/wkr done: free their SBUF and start the first z loads
        p_wk.release()
        p_z_box[0] = tc.alloc_tile_pool(name="p2_z", bufs=1, side="right")
        state[0] = sec_a(0)
        state[1] = sec_a(1)
        state[2] = sec_a(2)

        # ---- Wz slice + c0 ----
        ps_z = [pp.tile([128, 512], F32, tag=f"m{k}", name=f"ps_z{k}")
                for k in range(8)]
        wz_stage = p_acc.tile([128, 2, D], BF16, tag="wzstage")
        cacc = p_acc.tile([128, 2, D], BF16, tag="cacc")
        for j in range(DC):
            wo_t = p_io.tile([128, D], BF16, tag="wot", bufs=3)
            eng = nc.scalar if j % 2 == 0 else nc.sync
            eng.dma_start(wo_t[:], woT[j * 128:(j + 1) * 128, :])
            for dd in range(2):
                for e in range(EC):
                    nc.tensor.matmul(
                        ps_z[dd * EC + e][:],
                        wv_sb[:, j, dd * 128:(dd + 1) * 128],
                        wo_t[:, e * 512:(e + 1) * 512],
                        start=(j == 0), stop=(j == DC - 1))
            if j < 2:
                nc.vector.tensor_scalar(cacc[:, j, :], wo_t[:],
                                        bv_col[:, j:j + 1], None, op0=MULT)
            else:
                ct = p_io.tile([128, D], BF16, tag="ct", bufs=2)
                nc.vector.tensor_scalar(ct[:], wo_t[:],
                                        bv_col[:, j:j + 1], None, op0=MULT)
                nc.vector.tensor_tensor(cacc[:, j % 2, :], cacc[:, j % 2, :],
                                        ct[:], op=ADD)
        for dd in range(2):
            for e in range(EC):
                nc.scalar.activation(wz_stage[:, dd, e * 512:(e + 1) * 512],
                                     ps_z[dd * EC + e][:], CPY)
        # AG2 issued as soon as wz_stage is staged (y needs it ~2 tiles in)
        nc.sync.dma_start(ag2_in.rearrange("(dd p) d -> p dd d", p=128),
                          wz_stage[:])
        nc.gpsimd.collective_compute(
            "AllGather", mybir.AluOpType.bypass, replica_groups=RG,
            ins=[ag2_in], outs=[ag2_out])

        # c0 = 3*(bv@woT) + 3*bo
        nc.vector.tensor_tensor(cacc[:, 0, :], cacc[:, 0, :], cacc[:, 1, :],
                                op=ADD)
        ps_c = [pp.tile([1, 512], F32, tag=f"m{k}", name=f"ps_c{k}")
                for k in range(4)]
        for e in range(EC):
            nc.tensor.matmul(ps_c[e][:], ones_128[:],
                             cacc[:, 0, e * 512:(e + 1) * 512],
                             start=True, stop=True)
        c0_f = p_acc.tile([1, D], F32, tag="c0f")
        for e in range(EC):
            nc.scalar.activation(c0_f[0:1, e * 512:(e + 1) * 512],
                                 ps_c[e][:], CPY, scale=3.0)
        nc.vector.tensor_scalar(bo_row[:], bo_row[:], 3.0, None, op0=MULT)
        nc.vector.tensor_tensor(c0_f[:], c0_f[:], bo_row[:], op=ADD)
        nc.vector.tensor_copy(c0_row[:], c0_f[:])

        # a/r rows from first gather half -> broadcast (after AG issues so
        # the gpsimd engine never delays a collective)
        for c in range(NCORES):
            nc.gpsimd.dma_start(a_row8[0:1, c * SH:(c + 1) * SH],
                                ag1a_out[c, 0:1, 0:SH])
            nc.gpsimd.dma_start(r_row8[0:1, c * SH:(c + 1) * SH],
                                ag1a_out[c, 1:2, 0:SH])
        ar8_rep = p_acc.tile([128, 2, D], F8, tag="ar8rep")
        nc.gpsimd.partition_broadcast(ar8_rep[:, 0, :], a_row8[:])
        nc.gpsimd.partition_broadcast(ar8_rep[:, 1, :], r_row8[:])
        nc.scalar.activation(a_rep[:], ar8_rep[:, 0, :], CPY, scale=1.0 / LAM)
        nc.scalar.activation(r_rep[:], ar8_rep[:, 1, :], CPY, scale=1.0 / LAM)
        nc.gpsimd.partition_broadcast(kap_col[:], kap_row[:])

    # gathered M~ into SBUF, chunk index = (half, core): global d-chunk
    # 2c+h lives at m_bf[:, h, c, :]
    wz0_pool = tc.alloc_tile_pool(name="wz0_pool", bufs=1, side="right")
    wz0 = wz0_pool.tile([128, DC, 512], BF16)        # Wz e-quarter 0
    m_pool = tc.alloc_tile_pool(name="m_pool", bufs=1, side="right")
    m_bf4 = m_pool.tile([128, 2, NCORES, D], F8)     # M~[d, e]
    nc.scalar.dma_start(m_bf4[:, 0, 0:4, :],
                        ag1a_out[0:4, 2:130, :].rearrange("c p d -> p c d"))
    nc.scalar.dma_start(m_bf4[:, 0, 4:8, :],
                        ag1a_out[4:8, 2:130, :].rearrange("c p d -> p c d"))
    nc.gpsimd.dma_start(m_bf4[:, 1, 0:4, :],
                        ag1b_out[0:4, :, :].rearrange("c p d -> p c d"))
    nc.gpsimd.dma_start(m_bf4[:, 1, 4:8, :],
                        ag1b_out[4:8, :, :].rearrange("c p d -> p c d"))
    m_bf = m_bf4.rearrange("p h c d -> p (h c) d")
    nc.gpsimd.dma_start(
        wz0[:],
        ag2_out[:, :, 0:512].rearrange("c (h p) e -> p (c h) e", p=128))

    p_wz_box = [None]
    wzq_tiles = {}

    def load_wzq(q):
        wzq = p_wz_box[0].tile([128, DC, 512], BF16, tag="wzq", bufs=2,
                               name="wzq")
        eng = nc.sync if q % 2 == 0 else nc.scalar
        eng.dma_start(
            wzq[:],
            ag2_out[:, :, q * 512:(q + 1) * 512]
            .rearrange("c (h p) e -> p (c h) e", p=128))
        return wzq

    # ---------------- Phase 2: per b-tile scores/softmax/zv ----------------
    with (
        tc.tile_pool(name="p2_g", bufs=1) as p_g,
        tc.tile_pool(name="p2_sc", bufs=1) as p_sc,
        tc.tile_pool(name="p2_io", bufs=1) as p_io2,
        tc.tile_pool(name="p2_psg", bufs=6, space="PSUM") as pp_g,
        tc.tile_pool(name="p2_psy", bufs=2, space="PSUM") as pp_y0,
    ):
        def sec_c(ib, st):
            """G~ = zq @ M~ on PE; gt = psum/LAM + r (DVE); dots"""
            sraw = p_sc.tile([128, T, T], F32, tag="sraw", bufs=2)
            st["sraw"] = sraw
            traw = p_sc.tile([128, 2], F32, tag="traw", bufs=2)
            st["traw"] = traw
            for t in range(T):
                gt = p_g.tile([128, D], BF16, tag="gt", bufs=3, name="gt")
                for e in range(EC):
                    ps = pp_g.tile([128, 512], F32)
                    for u in range(DC // 2):
                        nc.tensor.matmul(
                            ps[:], st["zqt"][:, t, 2 * u:2 * u + 2, :],
                            m_bf[:, 2 * u:2 * u + 2, e * 512:(e + 1) * 512],
                            start=(u == 0), stop=(u == DC // 2 - 1),
                            perf_mode=DR)
                    # gt = G/LAM + r  (score fold; G psum is LAM-scaled fp8)
                    nc.vector.scalar_tensor_tensor(
                        gt[:, e * 512:(e + 1) * 512], ps[:], 1.0 / LAM,
                        r_rep[:, e * 512:(e + 1) * 512],
                        op0=MULT, op1=ADD)
                # score dots for this t: (G+r).zk[s], s >= t
                for s in range(t, T):
                    scr = p_io2.tile([128, D], BF16, tag="scr", bufs=3)
                    nc.vector.tensor_tensor(scr[:], gt[:],
                                            st["zk"][:, s, :], op=MULT)
                    nc.scalar.activation(scr[:], scr[:], CPY,
                                         accum_out=sraw[:, t, s:s + 1])
                # a-dot for t=1,2 (row-0 softmax is shift-invariant)
                if t >= 1:
                    scr = p_io2.tile([128, D], BF16, tag="scr", bufs=3)
                    nc.vector.tensor_tensor(scr[:], st["zq"][:, t, :],
                                            a_rep[:], op=MULT)
                    nc.scalar.activation(scr[:], scr[:], CPY,
                                         accum_out=st["traw"][:, t - 1:t])

        def sec_b(ib, st):
            """softmax + zv + zv^T round trip (DVE/ACT + DMA XBAR)"""
            sraw = st["sraw"]
            tvec = p_sc.tile([128, 2], F32, tag="tvec", bufs=1)
            nc.vector.tensor_scalar(tvec[:], st["traw"][:], 1.0, kap_col[:],
                                    op0=MULT, op1=ADD)
            # softmax; masked entries = exp(0) = 1; row 0 needs no bias
            p_un = p_sc.tile([128, T, T], F32, tag="p_un", bufs=1)
            nc.scalar.activation(p_un[:, 0, :], sraw[:, 0, :], EXP)
            nc.scalar.activation(p_un[:, 1, 1:], sraw[:, 1, 1:], EXP,
                                 bias=tvec[:, 0:1])
            nc.scalar.activation(p_un[:, 2, 2:], sraw[:, 2, 2:], EXP,
                                 bias=tvec[:, 1:2])
            nc.vector.memset(p_un[:, 1, 0:1], 1.0)
            nc.vector.memset(p_un[:, 2, 0:2], 1.0)
            rsum = p_sc.tile([128, T], F32, tag="rsum", bufs=1)
            nc.vector.tensor_reduce(rsum[:], p_un[:],
                                    axis=mybir.AxisListType.X, op=ADD)
            rinv = p_sc.tile([128, T], F32, tag="rinv", bufs=1)
            nc.vector.reciprocal(rinv[:], rsum[:])
            pn = p_sc.tile([128, T, T], F32, tag="pn", bufs=1)
            for t in range(T):
                nc.vector.tensor_scalar(pn[:, t, :], p_un[:, t, :],
                                        rinv[:, t:t + 1], None, op0=MULT)
            ws = p_sc.tile([128, T], F32, tag="ws", bufs=1)
            nc.vector.tensor_reduce(ws[:], pn.rearrange("p t s -> p s t"),
                                    axis=mybir.AxisListType.X, op=ADD)
            # zv = sum_s ws[s] * zq[s]
            zv_bf = p_sc.tile([128, D], BF16, tag="zv", bufs=2)
            zv_t1 = p_io2.tile([128, D], BF16, tag="scr", bufs=3)
            zv_t2 = p_io2.tile([128, D], BF16, tag="scr", bufs=3)
            nc.vector.tensor_scalar(zv_bf[:], st["zq"][:, 0, :], ws[:, 0:1],
                                    None, op0=MULT)
            nc.vector.tensor_scalar(zv_t1[:], st["zq"][:, 1, :], ws[:, 1:2],
                                    None, op0=MULT)
            nc.scalar.activation(zv_t2[:], st["zq"][:, 2, :], CPY,
                                 scale=ws[:, 2:3])
            nc.vector.tensor_tensor(zv_bf[:], zv_bf[:], zv_t1[:], op=ADD)
            nc.vector.tensor_tensor(zv_bf[:], zv_bf[:], zv_t2[:], op=ADD)
            # zv -> DRAM -> XBAR transpose back into zvT_all[:, :, ib tile]
            r0 = ib * 128
            nc.sync.dma_start(zv_dram[r0:r0 + 128, :], zv_bf[:])
            nc.scalar.dma_start_transpose(
                zvT_all[:, :, r0:r0 + 128], zv_dram[r0:r0 + 128, :])

        def y_q0(ib):
            ps = pp_y0.tile([128, 512], F32)
            for dc in range(DC):
                nc.tensor.matmul(
                    ps[:], zvT_all[:, dc, ib * 128:(ib + 1) * 128],
                    wz0[:, dc, :], start=(dc == 0), stop=False)
            nc.tensor.matmul(ps[:], ones_col[:], c0_row[0:1, 0:512],
                             start=False, stop=True)
            y_sb = p_sc.tile([128, 512], F32, tag="ysb", bufs=2)
            nc.scalar.activation(y_sb[:], ps[:], CPY)
            nc.sync.dma_start(out[ib * 128:(ib + 1) * 128, 0:512], y_sb[:])

        for ib in range(NB):
            if ib + 3 < NB:
                state[ib + 3] = sec_a(ib + 3)
            sec_c(ib, state[ib])
            sec_b(ib, state[ib])
            if ib >= 2:
                y_q0(ib - 2)
        # last G done: free M~'s SBUF and prefetch the first phase-4 wz
        # quarter while the trailing y_q0 tiles run
        m_pool.release()
        p_wz_box[0] = tc.alloc_tile_pool(name="p4_wz", bufs=1, side="right")
        wzq_tiles[1] = load_wzq(1)
        y_q0(NB - 2)
        y_q0(NB - 1)

    # ---------------- Phase 4: y quarters 1..3 -----------------------------
    with (
        tc.tile_pool(name="p4_y", bufs=3) as p_y,
        tc.tile_pool(name="p4_ps", bufs=4, space="PSUM") as pp_y,
    ):
        for q in range(1, EC):
            if q + 1 < EC:
                wzq_tiles[q + 1] = load_wzq(q + 1)
            wzq = wzq_tiles[q]
            for ib in range(NB):
                ps = pp_y.tile([128, 512], F32)
                for dc in range(DC):
                    nc.tensor.matmul(
                        ps[:], zvT_all[:, dc, ib * 128:(ib + 1) * 128],
                        wzq[:, dc, :],
                        start=(dc == 0), stop=False)
                nc.tensor.matmul(ps[:], ones_col[:],
                                 c0_row[0:1, q * 512:(q + 1) * 512],
                                 start=False, stop=True)
                y_sb = p_y.tile([128, 512], F32)
                nc.scalar.activation(y_sb[:], ps[:], CPY)
                nc.sync.dma_start(
                    out[ib * 128:(ib + 1) * 128, q * 512:(q + 1) * 512],
                    y_sb[:])

    p_wz_box[0].release()
    wz0_pool.release()
    p_z_box[0].release()
    zvT_pool.release()
    persist.release()
    const.release()


def build_nc(b_loc):
    nc = bacc.Bacc("TRN2", target_bir_lowering=False, debug=False,
                   num_devices=NCORES)
    NB = b_loc // 128
    aps = {}
    aps["z"] = nc.dram_tensor("z", [b_loc, 2 * T * D], BF16,
                              kind="ExternalInput").ap()
    aps["zqt8"] = nc.dram_tensor("zqt8", [NB, 128, T, DC, 128], F8,
                                 kind="ExternalInput").ap()
    aps["wq_s"] = nc.dram_tensor("wq_s", [D, SH], BF16,
                                 kind="ExternalInput").ap()
    aps["wk"] = nc.dram_tensor("wk", [D, D], BF16, kind="ExternalInput").ap()
    aps["wk_rs"] = nc.dram_tensor("wk_rs", [D, SH], BF16,
                                  kind="ExternalInput").ap()
    aps["wv_s"] = nc.dram_tensor("wv_s", [D, SH], BF16,
                                 kind="ExternalInput").ap()
    aps["woT"] = nc.dram_tensor("woT", [D, D], BF16, kind="ExternalInput").ap()
    for b_ in ("bq", "bk", "bv", "bo"):
        aps[b_] = nc.dram_tensor(b_, [D], F32, kind="ExternalInput").ap()
    aps["out"] = nc.dram_tensor("out", [b_loc, D], F32,
                                kind="ExternalOutput").ap()
    with tile.TileContext(nc) as tc:
        emit(tc, aps, b_loc)
    nc.compile()
    return nc


_CACHE = {}


def _get_nc(b_loc):
    if b_loc not in _CACHE:
        _CACHE[b_loc] = build_nc(b_loc)
    return _CACHE[b_loc]


def make_in_maps(arrs):
    """Host-side sharding/layout prep: bf16 casts, transposes, fp8 quant."""
    b_loc = B // NCORES
    NB = b_loc // 128
    z_bf = np.ascontiguousarray(arrs["z"]).astype(BF)
    wk_bf = np.ascontiguousarray(arrs["wk"]).astype(BF)
    woT_bf = np.ascontiguousarray(arrs["wo"].T).astype(BF)
    biases = {k: np.ascontiguousarray(arrs[k], dtype=np.float32)
              for k in ("bq", "bk", "bv", "bo")}
    perm = np.array(D_SLOT_GLOBAL)
    in_maps = []
    for c in range(NCORES):
        m = dict(biases)
        zc = z_bf[c * b_loc:(c + 1) * b_loc]
        m["z"] = zc
        # zqt8[ib, p, t, slot, j] = fp8(zq[ib*128+j, t, glob(slot)*128+p])
        zq5 = np.asarray(zc[:, :T * D]).reshape(NB, 128, T, DC, 128)
        zt = zq5.transpose(0, 4, 2, 3, 1)[:, :, :, perm, :]
        m["zqt8"] = np.ascontiguousarray(zt).astype(F8E)
        m["wk"] = wk_bf
        m["wk_rs"] = np.ascontiguousarray(wk_bf[:, c * SH:(c + 1) * SH])
        m["woT"] = woT_bf
        m["wq_s"] = np.ascontiguousarray(
            arrs["wq"][:, c * SH:(c + 1) * SH]).astype(BF)
        m["wv_s"] = np.ascontiguousarray(
            arrs["wv"][:, c * SH:(c + 1) * SH]).astype(BF)
        in_maps.append(m)
    return in_maps


def kernel(**inputs):
    arrs = {k: np.asarray(v) for k, v in inputs.items()}
    b_loc = B // NCORES
    nc = _get_nc(b_loc)
    in_maps = make_in_maps(arrs)
    res = run_bass_kernel_spmd(nc, in_maps, core_ids=list(range(NCORES)))
    return np.concatenate([np.asarray(r["out"]) for r in res.results], axis=0)
